# revision 57
# baseline (speedup 1.0000x reference)
"""Trainium2 Bass kernel for windowed (inverted-window) attention.

Problem: B=2, T=2048, C=2048, H=16 heads, D=128, WINDOW=512.
  q,k,v = x@Wq, x@Wk, x@Wv  (per-head reshape), RoPE on q,k,
  scores masked so positions INSIDE the causal window are masked out
  (attend only to j>i or j<i-511), softmax, o@Wo.

Sharding: 8 cores = 2 (batch) x 4 (head groups of 4 heads).
Each core computes its batch's 4 heads end-to-end plus a partial
output projection (row-chunk of Wo); host sums the 4 partials per batch.

Matmul operands are bf16 (fp32 PSUM accumulation); partial outputs are
written bf16 and summed f32 on host.

Schedule: single 8-bank PSUM pool shared by both phases. Phase A runs
K-projection before Q per t-block so K's RoPE drains under the Q/V
matmuls. Phase B is software-pipelined: slot n emits scores(n)
interleaved with AV/z of slot n-1 and the output projection of the
previous i-block, keeping the tensor queue dense (the PE p-state ramp
penalizes idle gaps). Score matmuls skip the fully-masked i-range of
each (i-block, j-chunk) pair (always edge-aligned, so no extra
instructions).
"""

import sys
from collections import deque

import numpy as np

for _p in ("/opt/trn_rl_repo",):
    if _p not in sys.path:
        sys.path.insert(0, _p)

import ml_dtypes  # noqa: E402

# If BASS_TRACE is set in the environment, run_bass_kernel_spmd imports
# antenv.axon_hooks, which this container does not ship. Register a stub
# so tracing degrades gracefully instead of crashing.
try:
    import antenv.axon_hooks  # noqa: F401
except ImportError:
    import types as _types

    _hooks = _types.ModuleType("antenv.axon_hooks")
    _hooks._hook = None
    _hooks.set_axon_ntff_profile_hook = lambda h: setattr(_hooks, "_hook", h)
    _hooks.get_axon_ntff_profile_hook = lambda: _hooks._hook
    sys.modules["antenv.axon_hooks"] = _hooks
    import antenv as _antenv

    _antenv.axon_hooks = _hooks
import concourse.bass as bass  # noqa: E402
import concourse.mybir as mybir  # noqa: E402
from concourse.bacc import Bacc  # noqa: E402
from concourse.tile import TileContext  # noqa: E402
from concourse.bass import ts, ds  # noqa: E402
from concourse.bass_utils import run_bass_kernel_spmd  # noqa: E402

B, T, C, H, D = 2, 2048, 2048, 16, 128
HL = 4                # heads per core
NCORES = 8
WINDOW = 512
ROPE_BASE = 10000.0
TB = 512              # i/t block size (matmul free dim)
NTB = T // TB         # 4
CK = C // 128         # 16 contraction chunks for projections
NTC = T // 128        # 16 j-chunks / t-chunks
MASK_OFF = 511        # mask strip offset: off = i0 - j0 + MASK_OFF
MASK_W = 1664
F32 = mybir.dt.float32
BF16 = mybir.dt.bfloat16
AF = mybir.ActivationFunctionType

MM_DT = BF16
NP_MM = ml_dtypes.bfloat16
SCALE = float(1.0 / np.sqrt(D))

_NC = None
TRACE = False
LAST_RESULT = None    # BassKernelResults of the most recent run (for test.py)


def _score_range(ib, c):
    """Computed i-range (block-relative, [lo,hi)) for scores of j-chunk c,
    i-block ib. The complement is fully inside the causal window (attn=0
    after masking), so its matmul columns are skipped. The skipped range
    is always a prefix or suffix, never interior."""
    lo = max(0, 128 * c + 127 - TB * ib)   # start of fully-masked i-range
    hi = min(TB, 128 * c + 512 - TB * ib)  # end (exclusive)
    if lo >= hi:
        return 0, TB                       # no fully-masked range
    if lo == 0:
        return hi, TB                      # masked prefix -> compute suffix
    if hi == TB:
        return 0, lo                       # masked suffix -> compute prefix
    return 0, TB                           # interior hole (cannot happen)


def _mask_off(ib, c):
    """(strip offset, i-subrange [mlo,mhi)) if this (ib, c) block overlaps
    the window; the mask is 1.0 outside [mlo,mhi) so the multiply is
    trimmed to that range."""
    dd = ib * TB - c * 128
    if not (-(WINDOW - 1) <= dd <= (WINDOW - 1) + 127):
        return None
    mlo = max(0, 128 * c - TB * ib)
    mhi = min(TB, 128 * c + (WINDOW - 1) + 127 + 1 - TB * ib)
    return dd + MASK_OFF, mlo, mhi


def build_nc():
    nc = Bacc()
    xT = nc.declare_dram_parameter("xT", [C, T], MM_DT, isOutput=False)
    wq = nc.declare_dram_parameter("wq", [C, HL * D], MM_DT, isOutput=False)
    wk = nc.declare_dram_parameter("wk", [C, HL * D], MM_DT, isOutput=False)
    wv = nc.declare_dram_parameter("wv", [C, HL * D], MM_DT, isOutput=False)
    wo = nc.declare_dram_parameter("wo", [HL * D, C], MM_DT, isOutput=False)
    cosx = nc.declare_dram_parameter("cosx", [128, T], MM_DT, isOutput=False)
    sinx = nc.declare_dram_parameter("sinx", [128, T], MM_DT, isOutput=False)
    maskm = nc.declare_dram_parameter("maskm", [128, MASK_W], MM_DT, isOutput=False)
    out = nc.declare_dram_parameter("out", [T, C], MM_DT, isOutput=True)

    xT_v = xT[:].rearrange("(co p) t -> p co t", p=128)   # [128, 16, T]
    wq_v = wq[:].rearrange("(co p) d -> p co d", p=128)   # [128, 16, 512]
    wk_v = wk[:].rearrange("(co p) d -> p co d", p=128)
    wv_v = wv[:].rearrange("(co p) d -> p co d", p=128)
    wo_v = wo[:].rearrange("(h p) c -> p h c", p=128)     # [128, 4, C]

    with TileContext(nc) as tc:
        with (
            tc.tile_pool(name="res", bufs=1) as res,
            tc.tile_pool(name="xbp", bufs=19) as xbp,
            tc.tile_pool(name="rop", bufs=1) as rop,
            tc.tile_pool(name="etp", bufs=2) as etp,
            tc.tile_pool(name="up", bufs=1) as up,
            tc.tile_pool(name="otp", bufs=2) as otp,
            tc.tile_pool(name="msc", bufs=1) as msc,
            tc.tile_pool(name="obp", bufs=3) as obp,
            tc.tile_pool(name="ps", bufs=1, space="PSUM") as psp,
        ):
            # ---- residents; chunked loads for wq/wk so matmuls start early
            wkt = res.tile([128, CK, HL * D], MM_DT, name="wkt")
            wqt = res.tile([128, CK, HL * D], MM_DT, name="wqt")
            # first x chunks + first wk group gate the very first matmuls:
            # they go at the head of their queues
            xbs0 = [
                xbp.tile([128, TB], MM_DT, tag="xb", name=f"xb0_{ck}")
                for ck in range(CK)
            ]
            nc.sync.dma_start(xbs0[0][:], xT_v[:, 0, ts(0, TB)])
            nc.scalar.dma_start(xbs0[1][:], xT_v[:, 1, ts(0, TB)])
            # first wk pair split out so the very first K matmuls unblock
            # as early as possible
            nc.sync.dma_start(wkt[:, ts(0, 2), :], wk_v[:, ts(0, 2), :])
            nc.sync.dma_start(wkt[:, ds(2, 2), :], wk_v[:, ds(2, 2), :])
            for g in range(1, 4):   # 4-chunk groups: fewer issues, early start
                nc.sync.dma_start(wkt[:, ts(g, 4), :], wk_v[:, ts(g, 4), :])
            for ck in range(2, CK):
                eng = nc.gpsimd if ck % 2 == 0 else nc.scalar
                eng.dma_start(xbs0[ck][:], xT_v[:, ck, ts(0, TB)])
            # wq group 0 on sync right behind wk: the scalar queue is busy
            # with x chunks and delivered it ~3us after Q's first matmul
            nc.sync.dma_start(wqt[:, ts(0, 4), :], wq_v[:, ts(0, 4), :])
            cosb = res.tile([128, T], MM_DT, name="cosb")
            nc.sync.dma_start(cosb[:], cosx[:])
            sinb = res.tile([128, T], MM_DT, name="sinb")
            nc.sync.dma_start(sinb[:], sinx[:])
            for g in range(1, 4):
                nc.scalar.dma_start(wqt[:, ts(g, 4), :], wq_v[:, ts(g, 4), :])
            wvt = res.tile([128, CK, HL * D], MM_DT, name="wvt")
            for g in range(4):
                nc.sync.dma_start(wvt[:, ts(g, 4), :], wv_v[:, ts(g, 4), :])
            maskb = res.tile([128, MASK_W], MM_DT, name="maskb")
            nc.sync.dma_start(maskb[:], maskm[:])
            wob = res.tile([128, HL, C], MM_DT, name="wob")
            nc.sync.dma_start(wob[:], wo_v[:])

            QT = res.tile([128, HL, T], MM_DT, name="QT")   # q transposed [d, t]
            KT = res.tile([128, HL, T], MM_DT, name="KT")
            V = res.tile([128, NTC, HL * D], MM_DT, name="V")  # v natural [t, hd]
            ones = res.tile([128, 128], MM_DT, name="ones")
            nc.vector.memset(ones[:], 1.0)
            # preload the scalar engine's Copy/Exp activation tables during
            # the initial DMA fill so the first real copy/exp doesn't stall
            scr = res.tile([128, 2], F32, name="scr")
            nc.scalar.copy(scr[:, 0:1], ones[:, 0:1])
            nc.scalar.activation(scr[:, 1:2], ones[:, 0:1], AF.Exp, scale=SCALE)
            # zero every et buffer once (gpsimd is idle in phase A): exp is
            # trimmed to sub-ranges, so hole columns must never contain
            # kernel-start SBUF garbage (a NaN would survive the mask's x0)
            for _g in range(2):
                for c in range(NTC):
                    t = etp.tile([128, TB], MM_DT, tag=f"et{c}", name=f"etz{_g}_{c}")
                    nc.gpsimd.memset(t[:], 0.0)

            def ps(tag, name):
                return psp.tile([128, TB], F32, tag=tag, name=name)

            # ---------- phase B building blocks ----------
            S_TAGS = ("b4", "b5", "b6")
            slot_ets = {}     # n -> list of 16 et APs
            slot_u = {}       # n -> lvl1 pair-sum APs
            slot_w = {}       # n -> lvl2 quad-sum APs (z matmul inputs)
            slot_ps = {}      # n -> (pso, psz) APs
            oT_tiles = {}     # ib -> [128, HL, TB] bf16 tile

            def oT_of(ib):
                if ib not in oT_tiles:
                    oT_tiles[ib] = otp.tile(
                        [128, HL, TB], MM_DT, tag="oT", name=f"oT{ib}"
                    )
                return oT_tiles[ib]

            def emit_score(n, c):
                ib, h = divmod(n, HL)
                # slot 0 runs at the phase-A tail with no filler matmuls, so
                # borrow b7 (no o-chain uses it until slot 5) as a 4th tag
                tag = ("b4", "b5", "b6", "b7")[c % 4] if n == 0 else S_TAGS[c % 3]
                st = ps(tag, f"s{n}_{c}")
                lo, hi = _score_range(ib, c)
                nc.tensor.matmul(
                    st[:, ds(lo, hi - lo)], KT[:, h, ts(c, 128)],
                    QT[:, h, ds(ib * TB + lo, hi - lo)],
                    start=True, stop=True,
                )
                return st

            def emit_exp_mask(n, c, st):
                ib, h = divmod(n, HL)
                et = etp.tile([128, TB], MM_DT, tag=f"et{c}", name=f"et{n}_{c}")
                # exp only the computed i-range; the hole columns hold old
                # bounded values (buffers are memset once at startup) and are
                # zeroed by the mask multiply below
                lo, hi = _score_range(ib, c)
                nc.scalar.activation(et[:, ds(lo, hi - lo)],
                                     st[:, ds(lo, hi - lo)], AF.Exp, scale=SCALE)
                m = _mask_off(ib, c)
                if m is not None:
                    off, mlo, mhi = m
                    veng = nc.vector
                    veng.tensor_mul(
                        et[:, ds(mlo, mhi - mlo)], et[:, ds(mlo, mhi - mlo)],
                        maskb[:, ds(off + mlo, mhi - mlo)],
                    )
                slot_ets.setdefault(n, [None] * NTC)[c] = et

            def emit_lvl1(n, k):
                # u[k] = et[2k] + et[2k+1] on vector (fresh output tile);
                # when a pair of u's is done, fold them on gpsimd (idle in
                # phase B) so z needs only 4 matmuls.
                ets = slot_ets[n]
                u = up.tile([128, TB], MM_DT, tag=f"u{k}", name=f"u{n}_{k}")
                nc.vector.tensor_add(u[:], ets[2 * k][:], ets[2 * k + 1][:])
                slot_u.setdefault(n, [None] * 8)[k] = u
                if k % 2 == 1:
                    us = slot_u[n]
                    w = up.tile([128, TB], MM_DT, tag=f"w{k // 2}",
                                name=f"w{n}_{k // 2}")
                    # gpsimd adds are ~1.2us and kept stalling the z matmuls;
                    # vector absorbs these within its slot budget
                    nc.vector.tensor_add(w[:], us[k - 1][:], us[k][:])
                    slot_w.setdefault(n, [None] * 4)[k // 2] = w

            def build_prev_queue(pn):
                """Tensor-op closures for slot pn's AV + z (run during slot pn+1)."""
                pib, ph = divmod(pn, HL)
                pso = ps("b0" if pn % 2 == 0 else "b1", f"po{pn}")
                psz = ps("b2", f"pz{pn}")
                slot_ps[pn] = (pso, psz)
                q = deque()
                ets = slot_ets[pn]
                # AV also skips each chunk's fully-masked i-range (attn is
                # exactly 0 there after masking). A hole-free chunk goes
                # first (start=True must cover every column) and last.
                full = [c for c in range(NTC) if _score_range(pib, c) == (0, TB)]
                holed = [c for c in range(NTC) if _score_range(pib, c) != (0, TB)]
                seq = [full[0]] + holed + full[1:]
                for idx, c in enumerate(seq):
                    lo, hi = _score_range(pib, c)
                    q.append(lambda c=c, lo=lo, hi=hi, idx=idx: nc.tensor.matmul(
                        pso[:, ds(lo, hi - lo)], V[:, c, ts(ph, D)],
                        ets[c][:, ds(lo, hi - lo)],
                        start=(idx == 0), stop=(idx == len(seq) - 1),
                    ))
                w = slot_w[pn]
                zq = deque()
                for k in range(4):
                    zq.append(lambda k=k: nc.tensor.matmul(
                        psz[:], ones[:], w[k][:],
                        start=(k == 0), stop=(k == 3),
                    ))
                return q, zq

            def emit_recip_nm(pn):
                pib, ph = divmod(pn, HL)
                pso, psz = slot_ps[pn]
                rz = msc.tile([128, TB], F32, tag="rz", name=f"rz{pn}")
                nc.vector.reciprocal_approx_fast(rz[:], psz[:])
                nc.vector.tensor_mul(oT_of(pib)[:, ph, :], pso[:], rz[:])
                del slot_ets[pn], slot_u[pn], slot_w[pn], slot_ps[pn]

            ocount = [0]

            def emit_ochain(oib, cb, tto, tags=("b3", "b7"), alt_copy=False):
                """Output projection chain: out[oib*TB+tto*128 :, cb*TB :]."""
                k = ocount[0]
                ocount[0] += 1
                oc = ps(tags[k % len(tags)], f"oc{oib}_{cb}_{tto}")
                oTt = oT_of(oib)
                for hh in range(HL):
                    nc.tensor.matmul(
                        oc[:], oTt[:, hh, ts(tto, 128)],
                        wob[:, hh, ds(cb * TB, TB)],
                        start=(hh == 0), stop=(hh == HL - 1),
                    )
                ob = obp.tile([128, TB], MM_DT, tag="ob", name=f"ob{oib}_{cb}_{tto}")
                if alt_copy and k % 2 == 0:
                    nc.scalar.copy(ob[:], oc[:])
                else:
                    nc.vector.tensor_copy(ob[:], oc[:])
                nc.sync.dma_start(
                    out[ds(oib * TB + tto * 128, 128), ds(cb * TB, TB)], ob[:]
                )

            # o-chain (cb, tto) pairs for one i-block, split over its three
            # h>=1 successor slots: 6 + 5 + 5.
            OCH_SPLIT = {1: 6, 2: 5, 3: 5}

            def phaseB_slot(n):
                ib, h = divmod(n, HL)
                workq, zq = build_prev_queue(n - 1)
                och = deque()
                if ib >= 1 and h >= 1:
                    base = sum(OCH_SPLIT[hh] for hh in range(1, h))
                    for k in range(base, base + OCH_SPLIT[h]):
                        och.append((ib - 1, k % 4, k // 4))
                for c in range(NTC):
                    st = emit_score(n, c)
                    emit_exp_mask(n, c, st)
                    if c % 2 == 1:
                        emit_lvl1(n, c // 2)
                    for _ in range(2):
                        if workq:
                            workq.popleft()()
                    if c == 2:
                        # w(n-1) is complete by the end of slot n-1, so z can
                        # drain early; this frees the w tags (bufs=1) before
                        # this slot's own lvl2 adds need them on vector
                        while zq:
                            zq.popleft()()
                    if c % 3 == 2 and och:
                        emit_ochain(*och.popleft(), alt_copy=True)
                while workq:
                    workq.popleft()()
                while och:
                    emit_ochain(*och.popleft(), alt_copy=True)
                emit_recip_nm(n - 1)

            # ---------- phase A: projections (+ slot 0 interleaved) ----------
            for tb in range(NTB):
                if tb == 0:
                    xbs = xbs0
                else:
                    # gpsimd is idle mid-phase-A and can run these triggers
                    # far ahead; the scalar queue is busy with RoPE copies
                    xbs = []
                    for ck in range(CK):
                        xb = xbp.tile([128, TB], MM_DT, tag="xb",
                                      name=f"xb{tb}_{ck}")
                        nc.gpsimd.dma_start(xb[:], xT_v[:, ck, ts(tb, TB)])
                        xbs.append(xb)

                def rope(psums, OUT, which):
                    raw = rop.tile([128, HL, TB], MM_DT, tag="raw",
                                   name=f"raw{which}{tb}")
                    for h in range(HL):
                        nc.scalar.copy(raw[:, h, :], psums[h][:])
                    sw = rop.tile([128, HL, TB], MM_DT, tag="sw",
                                  name=f"sw{which}{tb}")
                    nc.sync.dma_start(sw[0:64, :, :], raw[64:128, :, :])
                    nc.sync.dma_start(sw[64:128, :, :], raw[0:64, :, :])
                    for h in range(HL):
                        nc.vector.tensor_mul(sw[:, h, :], sw[:, h, :],
                                             sinb[:, ts(tb, TB)])
                        nc.vector.tensor_mul(raw[:, h, :], raw[:, h, :],
                                             cosb[:, ts(tb, TB)])
                        nc.vector.tensor_add(OUT[:, h, ts(tb, TB)],
                                             sw[:, h, :], raw[:, h, :])

                # per-head chains: bank h is released (copied) as soon as its
                # 16-step chain ends, so the next consumer of the bank never
                # waits long, and RoPE starts 3 chains earlier
                kps = [ps(f"b{h}", f"pk{tb}_{h}") for h in range(HL)]
                if tb == 0:
                    # ck-outer: chunk ck first needed at matmul 4*ck, matching
                    # the rate the initial x DMAs land at
                    for ck in range(CK):
                        for h in range(HL):
                            nc.tensor.matmul(
                                kps[h][:], wkt[:, ck, ts(h, D)], xbs[ck][:],
                                start=(ck == 0), stop=(ck == CK - 1),
                            )
                else:
                    for h in range(HL):
                        for ck in range(CK):
                            nc.tensor.matmul(
                                kps[h][:], wkt[:, ck, ts(h, D)], xbs[ck][:],
                                start=(ck == 0), stop=(ck == CK - 1),
                            )
                rope(kps, KT, "k")
                qps = [ps(f"b{4 + h}", f"pq{tb}_{h}") for h in range(HL)]
                for h in range(HL):
                    for ck in range(CK):
                        nc.tensor.matmul(
                            qps[h][:], wqt[:, ck, ts(h, D)], xbs[ck][:],
                            start=(ck == 0), stop=(ck == CK - 1),
                        )
                rope(qps, QT, "q")
                # all four V chains advance together over reversed ck, so
                # every x chunk is released in the first few us of V and the
                # next t-block's x prefetch can run far ahead
                pvs = [ps(f"b{tco}", f"pv{tb * NTB + tco}") for tco in range(NTB)]
                for i, ck in enumerate(reversed(range(CK))):
                    for tco in range(NTB):
                        nc.tensor.matmul(
                            pvs[tco][:], xbs[ck][:, ts(tco, 128)], wvt[:, ck, :],
                            start=(i == 0), stop=(i == CK - 1),
                        )
                if tb == NTB - 1:
                    # slot-0 scores run at the tail of phase A; the first
                    # four exps go ahead of the V copies in the scalar queue
                    # so the 4-tag S rotation never waits on them
                    for c in range(4):
                        st = emit_score(0, c)
                        emit_exp_mask(0, c, st)
                        if c % 2 == 1:
                            emit_lvl1(0, c // 2)
                    for tco in range(NTB):
                        nc.scalar.copy(V[:, tb * NTB + tco, :], pvs[tco][:])
                    for c in range(4, NTC):
                        st = emit_score(0, c)
                        emit_exp_mask(0, c, st)
                        if c % 2 == 1:
                            emit_lvl1(0, c // 2)
                else:
                    for tco in range(NTB):
                        nc.scalar.copy(V[:, tb * NTB + tco, :], pvs[tco][:])

            # ---------- phase B: slots 1..15 + drain ----------
            for n in range(1, NTB * HL):
                phaseB_slot(n)
            workq, zq = build_prev_queue(NTB * HL - 1)
            while workq:
                workq.popleft()()
            while zq:
                zq.popleft()()
            emit_recip_nm(NTB * HL - 1)
            # drain: S banks are free, so rotate 4 oc banks and split the
            # PSUM->SBUF copies across scalar (idle) and vector
            for k in range(16):
                emit_ochain(NTB - 1, k % 4, k // 4,
                            tags=("b3", "b7", "b4", "b5", "b6", "b2"),
                            alt_copy=True)

    nc.finalize()
    return nc


def _host_tables():
    inv_freq = (
        1.0 / (np.float32(ROPE_BASE) ** (np.arange(0, D, 2, dtype=np.float32) / np.float32(D)))
    ).astype(np.float32)
    t = np.arange(T, dtype=np.float32)
    freqs = (t[:, None] * inv_freq[None, :]).astype(np.float32)  # [T, 64]
    cos = np.cos(freqs).T.astype(np.float32)                     # [64, T]
    sin = np.sin(freqs).T.astype(np.float32)
    cosx = np.ascontiguousarray(np.concatenate([cos, cos], axis=0)).astype(NP_MM)
    sinx = np.ascontiguousarray(np.concatenate([-sin, sin], axis=0)).astype(NP_MM)
    p = np.arange(128, dtype=np.int64)[:, None]
    u = np.arange(MASK_W, dtype=np.int64)[None, :]
    delta = u - MASK_OFF - p          # = i - j for tile offset
    allow = ~((delta >= 0) & (delta <= WINDOW - 1))
    maskm = np.ascontiguousarray(allow.astype(NP_MM))
    return cosx, sinx, maskm


def kernel(x, Wq, Wk, Wv, Wo):
    global _NC, LAST_RESULT
    if _NC is None:
        _NC = build_nc()
    x = np.asarray(x, dtype=np.float32)
    Wq = np.asarray(Wq, dtype=np.float32)
    Wk = np.asarray(Wk, dtype=np.float32)
    Wv = np.asarray(Wv, dtype=np.float32)
    Wo = np.asarray(Wo, dtype=np.float32)
    cosx, sinx, maskm = _host_tables()
    in_maps = []
    for core in range(NCORES):
        b, hg = divmod(core, NCORES // B)
        sl = slice(hg * HL * D, (hg + 1) * HL * D)
        in_maps.append(
            {
                "xT": np.ascontiguousarray(x[b].T.astype(NP_MM)),
                "wq": np.ascontiguousarray(Wq[:, sl].astype(NP_MM)),
                "wk": np.ascontiguousarray(Wk[:, sl].astype(NP_MM)),
                "wv": np.ascontiguousarray(Wv[:, sl].astype(NP_MM)),
                "wo": np.ascontiguousarray(Wo[sl, :].astype(NP_MM)),
                "cosx": cosx,
                "sinx": sinx,
                "maskm": maskm,
            }
        )
    res = run_bass_kernel_spmd(_NC, in_maps, list(range(NCORES)), trace=TRACE)
    LAST_RESULT = res
    out = np.zeros((B, T, C), dtype=np.float32)
    for core in range(NCORES):
        b = core // (NCORES // B)
        out[b] += res.results[core]["out"].astype(np.float32)
    return out
